# revision 27
# baseline (speedup 1.0000x reference)
"""Data-dependent ALiBi bias kernel for Trainium2, distributed over 8 NeuronCores.

Reference computation (per full input):
    logits = einsum('bnd,hd->bhn', x, W) + b          # [2, 16, 2048]
    fg     = log_sigmoid(logits)                      # [2, 16, 2048]
    fg     = cumsum(fg, axis=-1)
    out    = fg[:, :, :, None] - fg[:, :, None, :]    # [2, 16, 2048, 2048]

Sharding: 32 (batch, head) pairs / 8 cores = 4 heads per core, batch-major.
Each core computes its own [4, 2048, 2048] slab independently; no collectives.

v6 design (fp16 output stream at the DMA roofline):
  - Front pipeline, segmented in 4 x 512 sequence chunks: x^T seg DMA
    (1 MB contiguous fp16, host pre-arranged partition-major) -> PE matmul
    (PSUM accumulate over 8 d-chunks) -> ACT exp+ln (one explicit load of
    the combined natural_log_exp_and_others table during the input DMA
    wait; the framework's table-load pass then adds nothing) -> DVE
    carry-chained cumsum -> PE even/odd strided transposes -> gpsimd
    partition_broadcast, h-major so head 0 unblocks tile generation first.
    Chains are software-pipelined so each in-order engine rolls from
    segment to segment; the output stream starts ~33 us.
  - Output tiles pack TWO consecutive rows per partition: [128, 2, 2048]
    fp16 = 8 KB contiguous per partition in DRAM (8 KB descriptors sustain
    ~418 GB/s aggregate vs ~345 GB/s at 4 KB; per-queue rate caps at
    ~26 GB/s so larger tiles gain nothing). Row 2p+r of a 256-row chunk
    lives at partition p, slot r; the per-(p, r) bias -g[i] comes from PE
    transposes of stride-2 column slices of g. One dma_start per 1 MB tile
    (more, smaller dma_starts throttle on SP descriptor generation at
    ~0.9 us each; fewer, bigger tiles starve the ring FIFOs at the tail).
  - Each tile's two elementwise ops run on one engine, chosen greedily:
    ScalarE ACT Identity+bias (~3.9 us/tile) or VectorE tensor_scalar_add
    (~2.6 us/tile); deep per-engine tile pools (5+6 bufs) keep
    buffer-reuse WARs off the critical path so combined generation
    (~0.61 MB/us) stays ahead of the DMA roofline (~0.42 MB/us).
  - Host upcasts fp16 -> fp32 on gather; fp16 rounding adds ~2e-4
    Frobenius rel err (gate 2e-2).

Hardware gotchas baked in: PE matmul/transpose and partition_broadcast
operands at base partition 0; PSUM never a DMA source; ACT stays on one
activation table set the whole kernel.
"""

import numpy as np

B = 2
NH = 16
N = 2048
D = 1024
NCORES = 8
HPC = (B * NH) // NCORES  # 4 (batch, head) pairs per core
P = 128
DC = D // P      # 8 contraction chunks
SW = 512         # segment width (= max matmul moving free dim)
NSEG = N // SW   # 4
RPT = 2          # rows per partition in an output tile
NCH2 = N // (P * RPT)  # 8 output row-chunks (256 rows each) per head

_CACHE = {}


def _build_nc():
    import concourse.bacc as bacc
    import concourse.mybir as mybir
    from concourse.masks import make_identity
    from concourse.tile import TileContext

    f32 = mybir.dt.float32
    f16 = mybir.dt.float16
    f8 = mybir.dt.float8e4
    Act = mybir.ActivationFunctionType
    nc = bacc.Bacc(None, target_bir_lowering=False)

    # xT host-pre-arranged seg-major/partition-major:
    # xT[s, p, c, j] = x^T[c*128+p, s*512+j]
    xT = nc.dram_tensor("xT", [NSEG, P, DC, SW], f16, kind="ExternalInput")
    Wt = nc.dram_tensor("Wt", [D, HPC], f16, kind="ExternalInput")
    bv = nc.dram_tensor("bv", [HPC, 1], f32, kind="ExternalInput")
    out = nc.dram_tensor("out", [HPC, N, N], f16, kind="ExternalOutput")
    # view row i = c2*256 + 2p + r at [h, c2, p, r, :]
    out_r = out.rearrange("h (c2 p r) n -> h c2 p r n", p=P, r=RPT)

    with TileContext(nc) as tc:
        with (
            tc.tile_pool(name="big", bufs=1) as big,
            tc.tile_pool(name="small", bufs=1) as small,
            tc.tile_pool(name="useg", bufs=2) as usegp,
            tc.tile_pool(name="grp", bufs=3) as grp,
            tc.tile_pool(name="outa", bufs=5) as outa,
            tc.tile_pool(name="outv", bufs=5) as outv,
        ):
            ph1 = tc.tile_pool(name="ph1ps", bufs=3, space="PSUM")
            lps = ph1.__enter__()
            gpscm = tc.tile_pool(name="gps", bufs=2, space="PSUM")
            gps = gpscm.__enter__()

            # ---- inputs -> SBUF. Wt first (so ldweights never waits on it);
            # x^T per segment: 0.5 MB contiguous, 4 KB runs per partition.
            Wt_s = small.tile([P, DC, HPC], f16, tag="Wt")
            nc.sync.dma_start(out=Wt_s, in_=Wt.rearrange("(c p) h -> p c h", p=P))
            b_s = small.tile([HPC, 1], f32, tag="b")
            nc.sync.dma_start(out=b_s, in_=bv[:])
            xT_s = big.tile([P, NSEG, DC, SW], f16, tag="xT")
            for s in range(NSEG):
                nc.sync.dma_start(out=xT_s[:, s], in_=xT[s])
            nb = small.tile([HPC, 1], f32, tag="nb")
            nc.vector.tensor_scalar_mul(nb, b_s, -1.0)
            # one explicit load of the combined exp+ln+identity table, issued
            # while the x^T DMA streams
            ACT_SET_LN_EXP = 6  # natural_log_exp_and_others in act_info.json
            nc.scalar.add_instruction(
                mybir.InstLoadActFuncSet(
                    name=f"I-{nc.next_id()}",
                    act_func_set_id=ACT_SET_LN_EXP,
                    engine=mybir.EngineType.Activation,
                )
            )

            ident = small.tile([HPC, HPC], f32, tag="ident")
            make_identity(nc, ident)
            zeros = small.tile([HPC, SW], f32, tag="zeros")
            nc.gpsimd.memset(zeros, 0.0)

            g = small.tile([HPC, N], f32, tag="g")
            # ngEO[p, r, c2*HPC + h] = -g[h, c2*256 + 2p + r]
            ngEO = small.tile([P, RPT, NCH2 * HPC], f32, tag="ngEO")
            bcast = big.tile([P, HPC, N], f32, tag="bcast")

            ps_tiles = {}
            growt = {}

            def chain_mm(s):
                # logits^T [4, 512] for segment s, accumulated over c in PSUM
                ps = lps.tile([HPC, SW], f32, tag="lps")
                ps_tiles[s] = ps
                for c in range(DC):
                    nc.tensor.matmul(
                        ps,
                        Wt_s[:, c, :],
                        xT_s[:, s, c, :],
                        start=(c == 0),
                        stop=(c == DC - 1),
                    )

            def chain_post(s):
                lo, hi = s * SW, (s + 1) * SW
                ps = ps_tiles.pop(s)
                us = usegp.tile([HPC, SW], f32, tag="useg")
                # t = exp(-(logits + b)); u = ln(1 + t)
                nc.scalar.activation(us, ps, Act.Exp, bias=nb[:, 0:1], scale=-1.0)
                nc.scalar.activation(us, us, Act.Ln, bias=1.0)
                # g[:, lo:hi] = cumsum(useg) carried from the previous segment
                init = 0.0 if s == 0 else g[:, lo - 1 : lo]
                nc.vector.tensor_tensor_scan(
                    g[:, lo:hi], us, zeros, init,
                    mybir.AluOpType.add, mybir.AluOpType.add,
                )
                # per-(partition, row-slot) biases for the two 256-row chunks
                # this segment unlocks: transpose stride-2 column slices
                for c2 in (2 * s, 2 * s + 1):
                    base = c2 * RPT * P
                    for r in range(RPT):
                        gp = gps.tile([P, HPC], f32, tag="gps")
                        nc.tensor.transpose(
                            gp, g[:, base + r : base + RPT * P : RPT], ident
                        )
                        nc.vector.tensor_scalar_mul(
                            ngEO[:, r, c2 * HPC : (c2 + 1) * HPC], gp, -1.0
                        )
                # stage head rows 1-3 at partition 0 for the broadcasts.
                # The copy deliberately extends to the end of g: the extra
                # bytes are unused, but the read then depends on the LAST
                # segment's scan, so the list scheduler cannot queue these
                # heads' broadcasts on gpsimd ahead of (h0, s3) — which
                # otherwise runs ~4 broadcasts (~4.5 us) late and gates the
                # first output tile
                gend = min(N, (NSEG - 1) * SW + 64)
                wid = max(SW, gend - lo)
                for h in range(1, HPC):
                    grow = grp.tile([1, wid], f32, tag=f"grow{s}",
                                    name=f"grow{s}_{h}")
                    nc.sync.dma_start(out=grow, in_=g[h : h + 1, lo : lo + wid])
                    growt[(h, s)] = grow

            # software-pipelined front
            chain_mm(0)
            chain_mm(1)
            chain_post(0)
            chain_mm(2)
            chain_post(1)
            chain_mm(3)
            chain_post(2)
            chain_post(3)

            # bcast[p, h, :] = g[h, :], h-major so head 0 completes first and
            # tile generation (also h-major) starts as early as possible
            for h in range(HPC):
                for s in range(NSEG):
                    lo, hi = s * SW, (s + 1) * SW
                    src_row = (
                        g[0:1, lo:hi] if h == 0 else growt[(h, s)][:, 0:SW]
                    )
                    nc.gpsimd.partition_broadcast(bcast[:, h, lo:hi], src_row)

            # ---- output tiles [128, 2, 2048] fp16: row 2p+r of chunk c2 at
            # partition p slot r; two elementwise ops per tile, greedily
            # balanced between ScalarE (~3.9 us/tile) and VectorE (~2.6)
            eng_t = {"a": 0.0, "v": 0.0}
            for h in range(HPC):
                for c2 in range(NCH2):
                    col = c2 * HPC + h
                    if h == 0 and c2 < 2:
                        # first two tiles split across both engines so the
                        # first DMA fires ~2 us after the h0 broadcasts
                        ot = outa.tile([P, RPT, N], f16, tag="ot2a")
                        nc.scalar.activation(
                            ot[:, 0, :], bcast[:, h, :], Act.Identity,
                            bias=ngEO[:, 0, col : col + 1], scale=1.0,
                        )
                        nc.vector.tensor_scalar_add(
                            ot[:, 1, :], bcast[:, h, :],
                            ngEO[:, 1, col : col + 1],
                        )
                        eng_t["a"] += 1.95
                        eng_t["v"] += 1.31
                        nc.sync.dma_start(out=out_r[h, c2], in_=ot)
                        continue
                    use_a = eng_t["a"] + 3.90 < eng_t["v"] + 2.62
                    if use_a:
                        eng_t["a"] += 3.90
                        ot = outa.tile([P, RPT, N], f16, tag="ot2a")
                    else:
                        eng_t["v"] += 2.62
                        ot = outv.tile([P, RPT, N], f16, tag="ot2v")
                    for r in range(RPT):
                        if use_a:
                            nc.scalar.activation(
                                ot[:, r, :], bcast[:, h, :], Act.Identity,
                                bias=ngEO[:, r, col : col + 1], scale=1.0,
                            )
                        else:
                            nc.vector.tensor_scalar_add(
                                ot[:, r, :], bcast[:, h, :],
                                ngEO[:, r, col : col + 1],
                            )
                    nc.sync.dma_start(out=out_r[h, c2], in_=ot)

            gpscm.__exit__(None, None, None)
            ph1.__exit__(None, None, None)

    if not nc.is_finalized():
        nc.finalize()
    return nc


def _get_nc():
    if "nc" not in _CACHE:
        _CACHE["nc"] = _build_nc()
    return _CACHE["nc"]


def _make_in_maps(x, W, b):
    x = np.ascontiguousarray(x, dtype=np.float32)
    W = np.ascontiguousarray(W, dtype=np.float32)
    b = np.ascontiguousarray(b, dtype=np.float32)
    # seg-major partition-major: xT[s, p, c, j] = x[bi].T[c*128+p, s*512+j]
    xT_by_batch = [
        np.ascontiguousarray(
            x[bi].T.astype(np.float16)
            .reshape(DC, P, NSEG, SW)
            .transpose(2, 1, 0, 3)
        )
        for bi in range(B)
    ]
    in_maps = []
    for k in range(NCORES):
        bi = k // (NCORES // B)
        h0 = (k % (NCORES // B)) * HPC
        in_maps.append(
            {
                "xT": xT_by_batch[bi],
                "Wt": np.ascontiguousarray(W[h0 : h0 + HPC].T.astype(np.float16)),
                "bv": np.ascontiguousarray(b[h0 : h0 + HPC].reshape(HPC, 1)),
            }
        )
    return in_maps


def kernel(x, W, b, _trace=False, _trace_cores=None):
    from concourse.bass_utils import run_bass_kernel_spmd

    nc = _get_nc()
    in_maps = _make_in_maps(x, W, b)
    res = run_bass_kernel_spmd(
        nc, in_maps, core_ids=list(range(NCORES)), trace=_trace,
        trace_cores=_trace_cores,
    )
    _CACHE["last_results"] = res
    full = np.empty((B, NH, N, N), dtype=np.float32)
    for k in range(NCORES):
        bi = k // (NCORES // B)
        h0 = (k % (NCORES // B)) * HPC
        full[bi, h0 : h0 + HPC] = res.results[k]["out"]
    return full


# revision 31
# speedup vs baseline: 1.0332x; 1.0332x over previous
"""Data-dependent ALiBi bias kernel for Trainium2, distributed over 8 NeuronCores.

Reference computation (per full input):
    logits = einsum('bnd,hd->bhn', x, W) + b          # [2, 16, 2048]
    fg     = log_sigmoid(logits)                      # [2, 16, 2048]
    fg     = cumsum(fg, axis=-1)
    out    = fg[:, :, :, None] - fg[:, :, None, :]    # [2, 16, 2048, 2048]

Sharding: 32 (batch, head) pairs / 8 cores = 4 heads per core, batch-major.
Each core computes its own [4, 2048, 2048] slab independently; no collectives.

v6 design (fp16 output stream at the DMA roofline):
  - Front pipeline, segmented in 4 x 512 sequence chunks: x^T seg DMA
    (1 MB contiguous fp16, host pre-arranged partition-major) -> PE matmul
    (PSUM accumulate over 8 d-chunks) -> ACT exp+ln (one explicit load of
    the combined natural_log_exp_and_others table during the input DMA
    wait; the framework's table-load pass then adds nothing) -> DVE
    carry-chained cumsum -> PE even/odd strided transposes -> gpsimd
    partition_broadcast, h-major so head 0 unblocks tile generation first.
    Chains are software-pipelined so each in-order engine rolls from
    segment to segment; the output stream starts ~33 us.
  - Output tiles pack TWO consecutive rows per partition: [128, 2, 2048]
    fp16 = 8 KB contiguous per partition in DRAM (8 KB descriptors sustain
    ~418 GB/s aggregate vs ~345 GB/s at 4 KB; per-queue rate caps at
    ~26 GB/s so larger tiles gain nothing). Row 2p+r of a 256-row chunk
    lives at partition p, slot r; the per-(p, r) bias -g[i] comes from PE
    transposes of stride-2 column slices of g. One dma_start per 1 MB tile
    (more, smaller dma_starts throttle on SP descriptor generation at
    ~0.9 us each; fewer, bigger tiles starve the ring FIFOs at the tail).
  - Each tile's two elementwise ops run on one engine, chosen greedily:
    ScalarE ACT Identity+bias (~3.9 us/tile) or VectorE tensor_scalar_add
    (~2.6 us/tile); deep per-engine tile pools (5+6 bufs) keep
    buffer-reuse WARs off the critical path so combined generation
    (~0.61 MB/us) stays ahead of the DMA roofline (~0.42 MB/us).
  - Host upcasts fp16 -> fp32 on gather; fp16 rounding adds ~2e-4
    Frobenius rel err (gate 2e-2).

Hardware gotchas baked in: PE matmul/transpose and partition_broadcast
operands at base partition 0; PSUM never a DMA source; ACT stays on one
activation table set the whole kernel.
"""

import numpy as np

B = 2
NH = 16
N = 2048
D = 1024
NCORES = 8
HPC = (B * NH) // NCORES  # 4 (batch, head) pairs per core
P = 128
DC = D // P      # 8 contraction chunks
SW = 512         # segment width (= max matmul moving free dim)
NSEG = N // SW   # 4
RPT = 2          # rows per partition in an output tile
NCH2 = N // (P * RPT)  # 8 output row-chunks (256 rows each) per head

_CACHE = {}


def _build_nc():
    import concourse.bacc as bacc
    import concourse.mybir as mybir
    from concourse.masks import make_identity
    from concourse.tile import TileContext

    f32 = mybir.dt.float32
    f16 = mybir.dt.float16
    f8 = mybir.dt.float8e4
    Act = mybir.ActivationFunctionType
    nc = bacc.Bacc(None, target_bir_lowering=False)

    # xT host-pre-arranged seg-major/partition-major:
    # xT[s, p, c, j] = x^T[c*128+p, s*512+j]
    xT = nc.dram_tensor("xT", [NSEG, P, DC, SW], f16, kind="ExternalInput")
    Wt = nc.dram_tensor("Wt", [D, HPC], f16, kind="ExternalInput")
    bv = nc.dram_tensor("bv", [HPC, 1], f32, kind="ExternalInput")
    out = nc.dram_tensor("out", [HPC, N, N], f16, kind="ExternalOutput")
    # view row i = c2*256 + 2p + r at [h, c2, p, r, :]
    out_r = out.rearrange("h (c2 p r) n -> h c2 p r n", p=P, r=RPT)

    with TileContext(nc) as tc:
        with (
            tc.tile_pool(name="big", bufs=1) as big,
            tc.tile_pool(name="small", bufs=1) as small,
            tc.tile_pool(name="useg", bufs=2) as usegp,
            tc.tile_pool(name="grp", bufs=12) as grp,
            tc.tile_pool(name="outa", bufs=5) as outa,
            tc.tile_pool(name="outv", bufs=6) as outv,
        ):
            ph1 = tc.tile_pool(name="ph1ps", bufs=3, space="PSUM")
            lps = ph1.__enter__()
            gpscm = tc.tile_pool(name="gps", bufs=2, space="PSUM")
            gps = gpscm.__enter__()

            # ---- inputs -> SBUF. Wt first (so ldweights never waits on it);
            # x^T per segment: 0.5 MB contiguous, 4 KB runs per partition.
            Wt_s = small.tile([P, DC, HPC], f16, tag="Wt")
            nc.sync.dma_start(out=Wt_s, in_=Wt.rearrange("(c p) h -> p c h", p=P))
            b_s = small.tile([HPC, 1], f32, tag="b")
            nc.sync.dma_start(out=b_s, in_=bv[:])
            xT_s = big.tile([P, NSEG, DC, SW], f16, tag="xT")
            # two dma_starts per segment (c-chunk halves): engages 8 DMA
            # rings instead of 4 for the input stream, and each matmul
            # accumulation group starts when its half of the segment lands
            for s in range(NSEG):
                nc.sync.dma_start(
                    out=xT_s[:, s, 0 : DC // 2], in_=xT[s, :, 0 : DC // 2]
                )
                nc.sync.dma_start(
                    out=xT_s[:, s, DC // 2 :], in_=xT[s, :, DC // 2 :]
                )
            nb = small.tile([HPC, 1], f32, tag="nb")
            nc.vector.tensor_scalar_mul(nb, b_s, -1.0)
            # one explicit load of the combined exp+ln+identity table, issued
            # while the x^T DMA streams
            ACT_SET_LN_EXP = 6  # natural_log_exp_and_others in act_info.json
            nc.scalar.add_instruction(
                mybir.InstLoadActFuncSet(
                    name=f"I-{nc.next_id()}",
                    act_func_set_id=ACT_SET_LN_EXP,
                    engine=mybir.EngineType.Activation,
                )
            )

            ident = small.tile([HPC, HPC], f32, tag="ident")
            make_identity(nc, ident)
            zeros = small.tile([HPC, SW], f32, tag="zeros")
            nc.gpsimd.memset(zeros, 0.0)

            g = small.tile([HPC, N], f32, tag="g")
            # ngEO[p, r, c2*HPC + h] = -g[h, c2*256 + 2p + r]
            ngEO = small.tile([P, RPT, NCH2 * HPC], f32, tag="ngEO")
            bcast = big.tile([P, HPC, N], f32, tag="bcast")

            ps_tiles = {}
            growt = {}

            def chain_mm(s):
                # logits^T [4, 512] for segment s, accumulated over c in PSUM
                ps = lps.tile([HPC, SW], f32, tag="lps")
                ps_tiles[s] = ps
                for c in range(DC):
                    nc.tensor.matmul(
                        ps,
                        Wt_s[:, c, :],
                        xT_s[:, s, c, :],
                        start=(c == 0),
                        stop=(c == DC - 1),
                    )

            def chain_post(s):
                lo, hi = s * SW, (s + 1) * SW
                ps = ps_tiles.pop(s)
                us = usegp.tile([HPC, SW], f32, tag="useg")
                # t = exp(-(logits + b)); u = ln(1 + t)
                nc.scalar.activation(us, ps, Act.Exp, bias=nb[:, 0:1], scale=-1.0)
                nc.scalar.activation(us, us, Act.Ln, bias=1.0)
                # g[:, lo:hi] = cumsum(useg) carried from the previous segment
                init = 0.0 if s == 0 else g[:, lo - 1 : lo]
                nc.vector.tensor_tensor_scan(
                    g[:, lo:hi], us, zeros, init,
                    mybir.AluOpType.add, mybir.AluOpType.add,
                )
                # per-(partition, row-slot) biases for the two 256-row chunks
                # this segment unlocks: transpose stride-2 column slices
                for c2 in (2 * s, 2 * s + 1):
                    base = c2 * RPT * P
                    for r in range(RPT):
                        gp = gps.tile([P, HPC], f32, tag="gps")
                        nc.tensor.transpose(
                            gp, g[:, base + r : base + RPT * P : RPT], ident
                        )
                        nc.vector.tensor_scalar_mul(
                            ngEO[:, r, c2 * HPC : (c2 + 1) * HPC], gp, -1.0
                        )
                # stage head rows 1-3 at partition 0 for the broadcasts
                for h in range(1, HPC):
                    grow = grp.tile([1, SW], f32, tag="grow")
                    nc.sync.dma_start(out=grow, in_=g[h : h + 1, lo:hi])
                    growt[(h, s)] = grow

            # software-pipelined front
            chain_mm(0)
            chain_mm(1)
            chain_post(0)
            chain_mm(2)
            chain_post(1)
            chain_mm(3)
            chain_post(2)
            chain_post(3)

            # bcast[p, h, :] = g[h, :], h-major so head 0 completes first and
            # tile generation (also h-major) starts as early as possible
            for h in range(HPC):
                for s in range(NSEG):
                    lo, hi = s * SW, (s + 1) * SW
                    src_row = g[0:1, lo:hi] if h == 0 else growt[(h, s)]
                    nc.gpsimd.partition_broadcast(bcast[:, h, lo:hi], src_row)

            # ---- output tiles [128, 2, 2048] fp16: row 2p+r of chunk c2 at
            # partition p slot r; two elementwise ops per tile, greedily
            # balanced between ScalarE (~3.9 us/tile) and VectorE (~2.6)
            eng_t = {"a": 0.0, "v": 0.0}
            for h in range(HPC):
                for c2 in range(NCH2):
                    col = c2 * HPC + h
                    use_a = eng_t["a"] + 3.90 < eng_t["v"] + 2.62
                    if use_a:
                        eng_t["a"] += 3.90
                        ot = outa.tile([P, RPT, N], f16, tag="ot2a")
                    else:
                        eng_t["v"] += 2.62
                        ot = outv.tile([P, RPT, N], f16, tag="ot2v")
                    for r in range(RPT):
                        if use_a:
                            nc.scalar.activation(
                                ot[:, r, :], bcast[:, h, :], Act.Identity,
                                bias=ngEO[:, r, col : col + 1], scale=1.0,
                            )
                        else:
                            nc.vector.tensor_scalar_add(
                                ot[:, r, :], bcast[:, h, :],
                                ngEO[:, r, col : col + 1],
                            )
                    nc.sync.dma_start(out=out_r[h, c2], in_=ot)

            gpscm.__exit__(None, None, None)
            ph1.__exit__(None, None, None)

    if not nc.is_finalized():
        nc.finalize()
    return nc


def _get_nc():
    if "nc" not in _CACHE:
        _CACHE["nc"] = _build_nc()
    return _CACHE["nc"]


def _make_in_maps(x, W, b):
    x = np.ascontiguousarray(x, dtype=np.float32)
    W = np.ascontiguousarray(W, dtype=np.float32)
    b = np.ascontiguousarray(b, dtype=np.float32)
    # seg-major partition-major: xT[s, p, c, j] = x[bi].T[c*128+p, s*512+j]
    xT_by_batch = [
        np.ascontiguousarray(
            x[bi].T.astype(np.float16)
            .reshape(DC, P, NSEG, SW)
            .transpose(2, 1, 0, 3)
        )
        for bi in range(B)
    ]
    in_maps = []
    for k in range(NCORES):
        bi = k // (NCORES // B)
        h0 = (k % (NCORES // B)) * HPC
        in_maps.append(
            {
                "xT": xT_by_batch[bi],
                "Wt": np.ascontiguousarray(W[h0 : h0 + HPC].T.astype(np.float16)),
                "bv": np.ascontiguousarray(b[h0 : h0 + HPC].reshape(HPC, 1)),
            }
        )
    return in_maps


def kernel(x, W, b, _trace=False, _trace_cores=None):
    from concourse.bass_utils import run_bass_kernel_spmd

    nc = _get_nc()
    in_maps = _make_in_maps(x, W, b)
    res = run_bass_kernel_spmd(
        nc, in_maps, core_ids=list(range(NCORES)), trace=_trace,
        trace_cores=_trace_cores,
    )
    _CACHE["last_results"] = res
    full = np.empty((B, NH, N, N), dtype=np.float32)
    for k in range(NCORES):
        bi = k // (NCORES // B)
        h0 = (k % (NCORES // B)) * HPC
        full[bi, h0 : h0 + HPC] = res.results[k]["out"]
    return full


# revision 32
# speedup vs baseline: 1.1630x; 1.1257x over previous
"""Data-dependent ALiBi bias kernel for Trainium2, distributed over 8 NeuronCores.

Reference computation (per full input):
    logits = einsum('bnd,hd->bhn', x, W) + b          # [2, 16, 2048]
    fg     = log_sigmoid(logits)                      # [2, 16, 2048]
    fg     = cumsum(fg, axis=-1)
    out    = fg[:, :, :, None] - fg[:, :, None, :]    # [2, 16, 2048, 2048]

Sharding: 32 (batch, head) pairs / 8 cores = 4 heads per core, batch-major.
Each core computes its own [4, 2048, 2048] slab independently; no collectives.

v6 design (fp16 output stream at the DMA roofline):
  - Front pipeline, segmented in 4 x 512 sequence chunks: x^T seg DMA
    (1 MB contiguous fp16, host pre-arranged partition-major) -> PE matmul
    (PSUM accumulate over 8 d-chunks) -> ACT exp+ln (one explicit load of
    the combined natural_log_exp_and_others table during the input DMA
    wait; the framework's table-load pass then adds nothing) -> DVE
    carry-chained cumsum -> PE even/odd strided transposes -> gpsimd
    partition_broadcast, h-major so head 0 unblocks tile generation first.
    Chains are software-pipelined so each in-order engine rolls from
    segment to segment; the output stream starts ~33 us.
  - Output tiles pack TWO consecutive rows per partition: [128, 2, 2048]
    fp16 = 8 KB contiguous per partition in DRAM (8 KB descriptors sustain
    ~418 GB/s aggregate vs ~345 GB/s at 4 KB; per-queue rate caps at
    ~26 GB/s so larger tiles gain nothing). Row 2p+r of a 256-row chunk
    lives at partition p, slot r; the per-(p, r) bias -g[i] comes from PE
    transposes of stride-2 column slices of g. One dma_start per 1 MB tile
    (more, smaller dma_starts throttle on SP descriptor generation at
    ~0.9 us each; fewer, bigger tiles starve the ring FIFOs at the tail).
  - Each tile's two elementwise ops run on one engine, chosen greedily:
    ScalarE ACT Identity+bias (~3.9 us/tile) or VectorE tensor_scalar_add
    (~2.6 us/tile); deep per-engine tile pools (5+6 bufs) keep
    buffer-reuse WARs off the critical path so combined generation
    (~0.61 MB/us) stays ahead of the DMA roofline (~0.42 MB/us).
  - Host upcasts fp16 -> fp32 on gather; fp16 rounding adds ~2e-4
    Frobenius rel err (gate 2e-2).

Hardware gotchas baked in: PE matmul/transpose and partition_broadcast
operands at base partition 0; PSUM never a DMA source; ACT stays on one
activation table set the whole kernel.
"""

import numpy as np

B = 2
NH = 16
N = 2048
D = 1024
NCORES = 8
HPC = (B * NH) // NCORES  # 4 (batch, head) pairs per core
P = 128
DC = D // P      # 8 contraction chunks
SW = 512         # segment width (= max matmul moving free dim)
NSEG = N // SW   # 4
RPT = 2          # rows per partition in an output tile
NCH2 = N // (P * RPT)  # 8 output row-chunks (256 rows each) per head

_CACHE = {}


def _build_nc():
    import concourse.bacc as bacc
    import concourse.mybir as mybir
    from concourse.masks import make_identity
    from concourse.tile import TileContext

    f32 = mybir.dt.float32
    f16 = mybir.dt.float16
    f8 = mybir.dt.float8e4
    Act = mybir.ActivationFunctionType
    nc = bacc.Bacc(None, target_bir_lowering=False)

    # xT host-pre-arranged seg-major/partition-major:
    # xT[s, p, c, j] = x^T[c*128+p, s*512+j]
    xT = nc.dram_tensor("xT", [NSEG, P, DC, SW], f16, kind="ExternalInput")
    Wt = nc.dram_tensor("Wt", [D, HPC], f16, kind="ExternalInput")
    bv = nc.dram_tensor("bv", [HPC, 1], f32, kind="ExternalInput")
    out = nc.dram_tensor("out", [HPC, N, N], f16, kind="ExternalOutput")
    # view row i = c2*256 + 2p + r at [h, c2, p, r, :]
    out_r = out.rearrange("h (c2 p r) n -> h c2 p r n", p=P, r=RPT)

    with TileContext(nc) as tc:
        with (
            tc.tile_pool(name="big", bufs=1) as big,
            tc.tile_pool(name="small", bufs=1) as small,
            tc.tile_pool(name="useg", bufs=2) as usegp,
            tc.tile_pool(name="grp", bufs=12) as grp,
            tc.tile_pool(name="outa", bufs=5) as outa,
            tc.tile_pool(name="outv", bufs=6) as outv,
        ):
            ph1 = tc.tile_pool(name="ph1ps", bufs=3, space="PSUM")
            lps = ph1.__enter__()
            gpscm = tc.tile_pool(name="gps", bufs=2, space="PSUM")
            gps = gpscm.__enter__()

            # ---- inputs -> SBUF. Wt first (so ldweights never waits on it);
            # x^T per segment: 0.5 MB contiguous, 4 KB runs per partition.
            Wt_s = small.tile([P, DC, HPC], f16, tag="Wt")
            nc.sync.dma_start(out=Wt_s, in_=Wt.rearrange("(c p) h -> p c h", p=P))
            b_s = small.tile([HPC, 1], f32, tag="b")
            nc.sync.dma_start(out=b_s, in_=bv[:])
            xT_s = big.tile([P, NSEG, DC, SW], f16, tag="xT")
            for s in range(NSEG):
                nc.sync.dma_start(out=xT_s[:, s], in_=xT[s])
            nb = small.tile([HPC, 1], f32, tag="nb")
            nc.vector.tensor_scalar_mul(nb, b_s, -1.0)
            # one explicit load of the combined exp+ln+identity table, issued
            # while the x^T DMA streams
            ACT_SET_LN_EXP = 6  # natural_log_exp_and_others in act_info.json
            nc.scalar.add_instruction(
                mybir.InstLoadActFuncSet(
                    name=f"I-{nc.next_id()}",
                    act_func_set_id=ACT_SET_LN_EXP,
                    engine=mybir.EngineType.Activation,
                )
            )

            ident = small.tile([HPC, HPC], f32, tag="ident")
            make_identity(nc, ident)
            zeros = small.tile([HPC, SW], f32, tag="zeros")
            nc.gpsimd.memset(zeros, 0.0)

            g = small.tile([HPC, N], f32, tag="g")
            # ngEO[p, r, c2*HPC + h] = -g[h, c2*256 + 2p + r]
            ngEO = small.tile([P, RPT, NCH2 * HPC], f32, tag="ngEO")
            bcast = big.tile([P, HPC, N], f32, tag="bcast")

            ps_tiles = {}
            growt = {}

            def chain_mm(s):
                # logits^T [4, 512] for segment s, accumulated over c in PSUM
                ps = lps.tile([HPC, SW], f32, tag="lps")
                ps_tiles[s] = ps
                for c in range(DC):
                    nc.tensor.matmul(
                        ps,
                        Wt_s[:, c, :],
                        xT_s[:, s, c, :],
                        start=(c == 0),
                        stop=(c == DC - 1),
                    )

            def chain_post(s):
                lo, hi = s * SW, (s + 1) * SW
                ps = ps_tiles.pop(s)
                us = usegp.tile([HPC, SW], f32, tag="useg")
                # t = exp(-(logits + b)); u = ln(1 + t)
                nc.scalar.activation(us, ps, Act.Exp, bias=nb[:, 0:1], scale=-1.0)
                nc.scalar.activation(us, us, Act.Ln, bias=1.0)
                # g[:, lo:hi] = cumsum(useg) carried from the previous segment
                init = 0.0 if s == 0 else g[:, lo - 1 : lo]
                nc.vector.tensor_tensor_scan(
                    g[:, lo:hi], us, zeros, init,
                    mybir.AluOpType.add, mybir.AluOpType.add,
                )
                # per-(partition, row-slot) biases for the two 256-row chunks
                # this segment unlocks: transpose stride-2 column slices
                for c2 in (2 * s, 2 * s + 1):
                    base = c2 * RPT * P
                    for r in range(RPT):
                        gp = gps.tile([P, HPC], f32, tag="gps")
                        nc.tensor.transpose(
                            gp, g[:, base + r : base + RPT * P : RPT], ident
                        )
                        nc.vector.tensor_scalar_mul(
                            ngEO[:, r, c2 * HPC : (c2 + 1) * HPC], gp, -1.0
                        )
                # stage head rows 1-3 at partition 0 for the broadcasts
                for h in range(1, HPC):
                    grow = grp.tile([1, SW], f32, tag="grow")
                    nc.sync.dma_start(out=grow, in_=g[h : h + 1, lo:hi])
                    growt[(h, s)] = grow

            # software-pipelined front
            chain_mm(0)
            chain_mm(1)
            chain_post(0)
            chain_mm(2)
            chain_post(1)
            chain_mm(3)
            chain_post(2)
            chain_post(3)

            # bcast[p, h, :] = g[h, :], h-major so head 0 completes first and
            # tile generation (also h-major) starts as early as possible
            for h in range(HPC):
                for s in range(NSEG):
                    lo, hi = s * SW, (s + 1) * SW
                    src_row = g[0:1, lo:hi] if h == 0 else growt[(h, s)]
                    nc.gpsimd.partition_broadcast(bcast[:, h, lo:hi], src_row)

            # ---- output tiles [128, 2, 2048] fp16: row 2p+r of chunk c2 at
            # partition p slot r; two elementwise ops per tile, greedily
            # balanced between ScalarE (~3.9 us/tile) and VectorE (~2.6)
            eng_t = {"a": 0.0, "v": 0.0}
            for h in range(HPC):
                for c2 in range(NCH2):
                    col = c2 * HPC + h
                    use_a = eng_t["a"] + 3.90 < eng_t["v"] + 2.62
                    if use_a:
                        eng_t["a"] += 3.90
                        ot = outa.tile([P, RPT, N], f16, tag="ot2a")
                    else:
                        eng_t["v"] += 2.62
                        ot = outv.tile([P, RPT, N], f16, tag="ot2v")
                    for r in range(RPT):
                        if use_a:
                            nc.scalar.activation(
                                ot[:, r, :], bcast[:, h, :], Act.Identity,
                                bias=ngEO[:, r, col : col + 1], scale=1.0,
                            )
                        else:
                            nc.vector.tensor_scalar_add(
                                ot[:, r, :], bcast[:, h, :],
                                ngEO[:, r, col : col + 1],
                            )
                    nc.sync.dma_start(out=out_r[h, c2], in_=ot)

            gpscm.__exit__(None, None, None)
            ph1.__exit__(None, None, None)

    if not nc.is_finalized():
        nc.finalize()
    return nc


def _get_nc():
    if "nc" not in _CACHE:
        _CACHE["nc"] = _build_nc()
    return _CACHE["nc"]


def _make_in_maps(x, W, b):
    x = np.ascontiguousarray(x, dtype=np.float32)
    W = np.ascontiguousarray(W, dtype=np.float32)
    b = np.ascontiguousarray(b, dtype=np.float32)
    # seg-major partition-major: xT[s, p, c, j] = x[bi].T[c*128+p, s*512+j]
    xT_by_batch = [
        np.ascontiguousarray(
            x[bi].T.astype(np.float16)
            .reshape(DC, P, NSEG, SW)
            .transpose(2, 1, 0, 3)
        )
        for bi in range(B)
    ]
    in_maps = []
    for k in range(NCORES):
        bi = k // (NCORES // B)
        h0 = (k % (NCORES // B)) * HPC
        in_maps.append(
            {
                "xT": xT_by_batch[bi],
                "Wt": np.ascontiguousarray(W[h0 : h0 + HPC].T.astype(np.float16)),
                "bv": np.ascontiguousarray(b[h0 : h0 + HPC].reshape(HPC, 1)),
            }
        )
    return in_maps


def kernel(x, W, b, _trace=False, _trace_cores=None):
    from concourse.bass_utils import run_bass_kernel_spmd

    nc = _get_nc()
    in_maps = _make_in_maps(x, W, b)
    res = run_bass_kernel_spmd(
        nc, in_maps, core_ids=list(range(NCORES)), trace=_trace,
        trace_cores=_trace_cores,
    )
    _CACHE["last_results"] = res
    full = np.empty((B, NH, N, N), dtype=np.float32)
    for k in range(NCORES):
        bi = k // (NCORES // B)
        h0 = (k % (NCORES // B)) * HPC
        full[bi, h0 : h0 + HPC] = res.results[k]["out"]
    return full
